# revision 2
# baseline (speedup 1.0000x reference)
"""Trainium2 Bass kernel for nn_Discriminator (batched bilinear form).

scores[b] = features[b] . (summary[b] @ weight.T)   for b in [0, 131072)

Strategy: data-parallel over 8 NeuronCores (batch sharded, weight replicated),
with all heavy inputs cast to bf16 and pre-packed on the host into
block-contiguous SBUF-ready layouts:
  - f_flat[p, :]  holds features rows (t*128+p) packed per DMA block
  - s_flat[p, :]  holds summary^T packed per DMA block, k-chunk-major
  - wt = weight.T [512, 512] bf16
Each DMA block is one contiguous >=2 KiB run per partition (max descriptor
efficiency). Per 128-row batch tile, the PE runs 4 accumulating bf16 matmuls
(stationary = S^T chunk [128k, 128b], moving = wt chunk [128k, 512h])
producing ws = S @ W^T directly in PSUM -- no on-device transposes at all.
DVE scalar_tensor_tensor fuses multiply+reduce: scores col = sum(F * ws).
Scores accumulate untransposed as sacc[p, t] and ship in ONE 64 KiB DMA at
the end; the host does the (t, p) unpermute for free.
bf16 halves HBM traffic (32.5 MiB/core) so DMA hides under the PE matmul
stream (512 MMs x ~216 ns ~= 111 us). Accumulation stays fp32 (PSUM + DVE);
rel err ~3e-3 vs the 2e-2 gate. Block schedule tapers at both ends so the
pipeline fills fast and drains fast.
"""

import numpy as np
import ml_dtypes

BF16 = np.dtype(ml_dtypes.bfloat16)

B = 131072
H = 512
NCORES = 8
BC = B // NCORES      # rows per core (16384)
P = 128               # partitions
T = BC // P           # batch tiles per core (128)
CHUNKS = H // P       # k-chunks (4)
NB = 8                # batch tiles per steady-state DMA block (1 MiB/stream)
FIRST_BLOCKS = (2, 2, 4)   # fast pipeline fill
LAST_BLOCKS = (4, 2, 2)    # fast pipeline drain
BUFS_S = 8            # S-block buffering depth
BUFS_F = 8            # F-block buffering depth
BUFS_PS = 6           # PSUM pool depth for ws tiles
BUFS_SCR = 3

_CACHE = {}


def _sched():
    """Block schedule [(tile0, ntiles), ...]; shared by host pack + kernel."""
    sched = []
    t0 = 0
    for s in FIRST_BLOCKS:
        sched.append((t0, s))
        t0 += s
    mid_end = T - sum(LAST_BLOCKS)
    while t0 < mid_end:
        sched.append((t0, min(NB, mid_end - t0)))
        t0 += min(NB, mid_end - t0)
    for s in LAST_BLOCKS:
        sched.append((t0, s))
        t0 += s
    assert t0 == T
    return sched


SCHED = _sched()
# free-axis element offset of each block in the flat [P, T*H] layouts
BLK_OFF = {}
_off = 0
for _t0, _sz in SCHED:
    BLK_OFF[_t0] = _off
    _off += _sz * H
assert _off == T * H


def _build():
    from concourse import bacc
    import concourse.mybir as mybir
    import concourse.tile as tile

    dt = mybir.dt
    nc = bacc.Bacc("TRN2", target_bir_lowering=False)

    W = T * H  # flat free width per partition (65536)
    f_flat = nc.dram_tensor("f_flat", [P, W], dt.bfloat16, kind="ExternalInput")
    s_flat = nc.dram_tensor("s_flat", [P, W], dt.bfloat16, kind="ExternalInput")
    wt = nc.dram_tensor("wt", [H, H], dt.bfloat16, kind="ExternalInput")
    scores_raw = nc.dram_tensor("scores_raw", [P, T], dt.float32,
                                kind="ExternalOutput")

    f_v = f_flat.ap()
    s_v = s_flat.ap()
    wt_v = wt.ap().rearrange("(c p) h -> p c h", p=P)   # [128, 4, 512]

    with tile.TileContext(nc) as tc:
        from contextlib import ExitStack
        with ExitStack() as ctx:
            singles = ctx.enter_context(tc.tile_pool(name="singles", bufs=1))
            sblocks = ctx.enter_context(tc.tile_pool(name="sblocks", bufs=BUFS_S))
            fblocks = ctx.enter_context(tc.tile_pool(name="fblocks", bufs=BUFS_F))
            scr = ctx.enter_context(tc.tile_pool(name="scr", bufs=BUFS_SCR))
            psW = ctx.enter_context(tc.tile_pool(name="psW", bufs=BUFS_PS, space="PSUM"))
            psWarm = ctx.enter_context(tc.tile_pool(name="psWarm", bufs=1, space="PSUM"))

            blk_of_tile = {}
            blk_size = {}
            for s0, sz in SCHED:
                blk_size[s0] = sz
                for tt in range(s0, s0 + sz):
                    blk_of_tile[tt] = s0
            # Head start: block 0's S rides the otherwise-idle ACT ring,
            # interleaved with wt (wt c0 first -- it gates MM #0), so the
            # first matmuls begin while the SP ring is still delivering
            # blocks 1+. Both rings stream S in parallel at startup.
            blk_cache = {}

            def emit_s(b0, eng):
                sz = blk_size[b0]
                off = BLK_OFF[b0]
                w = sz * H
                s_b = sblocks.tile([P, w], dt.bfloat16, name="s_blk", tag="s_blk")
                eng.dma_start(out=s_b[:], in_=s_v[:, off:off + w])
                return s_b

            def emit_f(b0):
                sz = blk_size[b0]
                off = BLK_OFF[b0]
                w = sz * H
                f_b = fblocks.tile([P, w], dt.bfloat16, name="f_blk", tag="f_blk")
                nc.sync.dma_start(out=f_b[:], in_=f_v[:, off:off + w])
                return f_b

            # HAM warmup: DVE memset (earliest-available writer, ~6.7us)
            # feeds 6 zero matmuls so most of the PE clock ramp burns in
            # preamble dead time instead of on the real stream.
            warm_src = singles.tile([P, H], dt.bfloat16, name="warm_src")
            nc.vector.memset(warm_src[:], 0.0)
            warm_ps = psWarm.tile([P, H], dt.float32, name="warm_ps", tag="warm")
            for _ in range(6):
                nc.tensor.matmul(warm_ps[:], warm_src[:, :P], warm_src[:],
                                 start=True, stop=True)

            wt_sb = singles.tile([P, CHUNKS, H], dt.bfloat16)
            nc.scalar.dma_start(out=wt_sb[:, 0, :], in_=wt_v[:, 0, :])
            s0_tile = emit_s(SCHED[0][0], nc.scalar)
            for c in range(1, CHUNKS):
                nc.scalar.dma_start(out=wt_sb[:, c, :], in_=wt_v[:, c, :])

            # SP ring: remaining S blocks with F trailing by one block:
            # S(1), S(2), F(0), S(3), F(1), ... Buffer WAR deps throttle it.
            starts = [s0 for s0, _ in SCHED]
            pend = {starts[0]: s0_tile}
            for j, b0 in enumerate(starts):
                if j >= 1:
                    pend[b0] = emit_s(b0, nc.sync)
                    pb = starts[j - 1]
                    blk_cache[pb] = (pend.pop(pb), emit_f(pb))
            blk_cache[starts[-1]] = (pend.pop(starts[-1]), emit_f(starts[-1]))

            def get_block(t):
                return blk_cache[blk_of_tile[t]]

            sacc = singles.tile([P, T], dt.float32, name="sacc")

            for t in range(T):
                s_b, f_b = get_block(t)
                b0 = blk_of_tile[t]
                sz = blk_size[b0]
                lb = t - b0

                # ws[b, h] = sum_k S[b, k] * W[h, k]: 4 accumulating matmuls,
                # stationary = S^T chunk (128x128 bf16), moving = wt chunk
                ps_w = psW.tile([P, H], dt.float32, name="ps_w", tag="ps_w")
                for c in range(CHUNKS):
                    o = (c * sz + lb) * P
                    nc.tensor.matmul(
                        ps_w[:],
                        s_b[:, o:o + P],
                        wt_sb[:, c, :],
                        start=(c == 0),
                        stop=(c == CHUNKS - 1),
                    )

                # scores[:, t] = sum_h F * ws  (fused multiply+reduce on DVE)
                stt_out = scr.tile([P, H], dt.float32, name="stt_out", tag="stt_out")
                nc.vector.scalar_tensor_tensor(
                    out=stt_out[:],
                    in0=f_b[:, lb * H:(lb + 1) * H],
                    scalar=1.0,
                    in1=ps_w[:],
                    op0=mybir.AluOpType.mult,
                    op1=mybir.AluOpType.mult,
                    accum_out=sacc[:, t:t + 1],
                )

            # one 64 KiB result DMA at the very end (host unpermutes)
            nc.sync.dma_start(out=scores_raw.ap(), in_=sacc[:])

    nc.finalize()
    return nc


def _get_nc():
    if "nc" not in _CACHE:
        _CACHE["nc"] = _build()
    return _CACHE["nc"]


def prep_in_maps(features, summary, weight):
    """Host-side shard + cast + block-pack.

    f_flat[p, off_j + u*H : off_j + (u+1)*H] = features[core*BC + (t0_j+u)*128 + p, :]
    s_flat[p, off_j + (c*sz_j + x)]          = summary^T chunk: S[b, k] with
                                               k = c*128 + p, b = (t0_j + x//128)*128...
    so each block j is one contiguous run per partition for both streams.
    """
    features = np.asarray(features, dtype=np.float32)
    summary = np.asarray(summary, dtype=np.float32)
    weight = np.asarray(weight, dtype=np.float32)

    fb = features.astype(BF16)                       # [B, H]
    sb = summary.astype(BF16)                        # [B, H]
    wt = np.ascontiguousarray(weight.T).astype(BF16)  # [H, H]

    maps = []
    for i in range(NCORES):
        fcore = fb[i * BC:(i + 1) * BC].reshape(T, P, H)     # [t, p, h]
        stcore = sb[i * BC:(i + 1) * BC].T.reshape(CHUNKS, P, BC)  # [c, p, b]
        fparts = []
        sparts = []
        for t0, sz in SCHED:
            fparts.append(
                fcore[t0:t0 + sz].transpose(1, 0, 2).reshape(P, sz * H))
            sparts.append(
                stcore[:, :, t0 * P:(t0 + sz) * P]
                .transpose(1, 0, 2).reshape(P, CHUNKS * sz * P))
        maps.append({
            "f_flat": np.ascontiguousarray(np.concatenate(fparts, axis=1)),
            "s_flat": np.ascontiguousarray(np.concatenate(sparts, axis=1)),
            "wt": wt,
        })
    return maps


def kernel(features, summary, weight):
    from concourse.bass_utils import run_bass_kernel_spmd

    nc = _get_nc()
    in_maps = prep_in_maps(features, summary, weight)
    res = run_bass_kernel_spmd(nc, in_maps, core_ids=list(range(NCORES)))
    # scores_raw[p, t] = scores[t*128 + p] per core -> transpose-gather
    return np.concatenate(
        [np.ascontiguousarray(r["scores_raw"].T).reshape(-1)
         for r in res.results])


if __name__ == "__main__":
    rng = np.random.default_rng(0)
    f = rng.standard_normal((B, H), dtype=np.float32)
    s = rng.standard_normal((B, H), dtype=np.float32)
    w = (rng.random((H, H), dtype=np.float32) - 0.5) * (2.0 / np.sqrt(H))
    got = kernel(f, s, w)
    want = ((s @ w.T) * f).sum(-1)
    err = np.abs(got - want)
    print("absmax-rel:", err.max() / np.abs(want).max())
